# revision 37
# baseline (speedup 1.0000x reference)
"""MultiHeadEMA on 8 Trainium2 NeuronCores.

Strategy
--------
Channel-sharded: embed_dim=1024 -> 8 slices of 128 channels (= SBUF
partitions), one per core. The reference's FFT conv is exactly an order-2 IIR
    y_n[l] = q_n y_n[l-1] + x[l],   out = silu(c0 y0 + c1 y1 + omega x)
computed with `tensor_tensor_scan` on the vector engine.

The DVE scan runs at ~2 cyc/elem, so the recurrence is decimated by 4:
    Y_n[j] = y_n[4j] satisfies  Y_n[j] = q_n^4 Y_n[j-1] + u_n[j]
    u_n[j] = x[4j] + q_n x[4j-1] + q_n^2 x[4j-2] + q_n^3 x[4j-3]
u_n is built by accumulating diagonal matmuls (tensor engine, bf16) into
PSUM from contiguous phase blocks of x (deinterleaved and pre-shifted on the
host). The scan reads u straight from PSUM at 1/4 length. Phase outputs
    pre_r = q0^r Y'_0 + q1^r Y'_1 + sum_s a_s x[4j+s]   (Y' = c_n-folded)

Engine split (perfetto-driven): PE matmuls issue at 216-259ns (N=512,
power-state dependent); DVE 1024-wide ops: tensor_scalar 4x ~478ns,
tensor_tensor 2x ~684ns, scan ~2280ns. Phase 0 (pre0 = w x0 + Y'0 + Y'1,
no PSUM accumulation needed) runs on DVE; phases 1-3 stay on PE
(x-pyramid + Y-terms into PSUM, Silu evacuates). PE 40 MMs/batch
(~8.6-10.4us) vs DVE ~7us/batch; moving more to DVE costs ~2x per
column (adds/stt are 1x-2x mode), so this split is the balance point.

Schedule: software pipelining (u(b+1) queued on PE before phases(b));
the 17 diagonal weight matrices are packed host-side (pure layout, like an
identity) and split into a u-part and a phase-part so the first matmul
isn't gated on the full 544KB; DMA rides three rings: sync and scalar
(HWDGE, ~100GB/s) carry the x slab halves + outputs, gpsimd (SWDGE,
slower) carries the weights + late slabs.
"""

import numpy as np
import ml_dtypes

import concourse.bass as bass
import concourse.bacc as bacc
import concourse.tile as tile
from concourse import mybir
from concourse.bass_utils import run_bass_kernel_spmd

SEQ_LEN, BSZ, EMBED_DIM, NDIM = 4096, 4, 1024, 2
N_CORES = 8
D_PER = EMBED_DIM // N_CORES  # 128 channels/core = full SBUF partitions
SCALE = (1.0 / NDIM) ** 0.5
DEC = 4                   # decimation factor
J = SEQ_LEN // DEC        # decimated length 1024
CH = 512                  # matmul chunk (one fp32 PSUM bank)
NG = J // CH              # j-groups per slab (2)
F32 = mybir.dt.float32
BF16 = mybir.dt.bfloat16
AF = mybir.ActivationFunctionType
ALU = mybir.AluOpType

# x phase blocks: r = 0..3 -> x[4j+r]. Shifted u-operands x[4j-k] are read
# as contiguous offset-(-1) views of block (4-k); only STRIDED rhs is slow.
NBLK = 4

NWU = 8   # u-stage diags: c_n q_n^k, k=0..3 (k-major)
NWP = 9   # phase diags: [q_n^r: r=1..3 (r-major)] (6), csum, cqs, cq2s
# coef columns: 0,1: q_n^4   2: omega   3,4: q_n   5: csum
NCOEF = 6


def build_bass():
    nc = bacc.Bacc(name="multihead_ema")
    x = nc.dram_tensor("x", [D_PER, BSZ, NBLK, J], BF16, kind="ExternalInput")
    wdu = nc.dram_tensor("wdu", [D_PER, NWU, D_PER], BF16, kind="ExternalInput")
    wdp = nc.dram_tensor("wdp", [D_PER, NWP, D_PER], BF16, kind="ExternalInput")
    coef = nc.dram_tensor("coef", [D_PER, NCOEF], F32, kind="ExternalInput")
    out = nc.dram_tensor("out", [D_PER, BSZ, DEC, J], BF16, kind="ExternalOutput")

    with tile.TileContext(nc) as tc:
        with (
            tc.tile_pool(name="const", bufs=1) as const,
            tc.tile_pool(name="xup", bufs=4) as xup,
            tc.tile_pool(name="yp", bufs=3) as yp,
            tc.tile_pool(name="ysp", bufs=2) as ysp,
            tc.tile_pool(name="op", bufs=4) as op,
            tc.tile_pool(name="psu", bufs=2, space="PSUM") as psu,
            tc.tile_pool(name="psc", bufs=2, space="PSUM") as psc,
        ):
            # DMA priority order: the k=0 u-weights land first on gpsimd
            # (they gate the first matmul); slab 0 is quartered per phase in
            # u-tap order (k: 0,1,2,3 reads phases 0,3,2,1) across the two
            # fast HWDGE rings so each tap's matmuls start as soon as its
            # phase lands; later slabs ride in halves.
            wusb = const.tile([D_PER, NWU, D_PER], BF16)
            nc.sync.dma_start(out=wusb[:, :, :], in_=wdu[:, :, :])
            xus = []
            for b in range(BSZ):
                xu = xup.tile([D_PER, 4, J], BF16, tag="xu")
                xus.append(xu)
            nc.sync.dma_start(out=xus[0][:, 0:1, :], in_=x[:, 0, 0:1, :])
            nc.scalar.dma_start(out=xus[0][:, 3:4, :], in_=x[:, 0, 3:4, :])
            nc.sync.dma_start(out=xus[0][:, 2:3, :], in_=x[:, 0, 2:3, :])
            nc.scalar.dma_start(out=xus[0][:, 1:2, :], in_=x[:, 0, 1:2, :])
            nc.sync.dma_start(out=xus[1][:, 0:2, :], in_=x[:, 1, 0:2, :])
            nc.scalar.dma_start(out=xus[1][:, 2:4, :], in_=x[:, 1, 2:4, :])
            csb = const.tile([D_PER, NCOEF], F32)
            nc.gpsimd.dma_start(out=csb[:, :], in_=coef[:, :])
            wpsb = const.tile([D_PER, NWP, D_PER], BF16)
            nc.gpsimd.dma_start(out=wpsb[:, :, :], in_=wdp[:, :, :])
            for b in (2, 3):
                nc.sync.dma_start(out=xus[b][:, 0:2, :], in_=x[:, b, 0:2, :])
                nc.gpsimd.dma_start(out=xus[b][:, 2:4, :], in_=x[:, b, 2:4, :])

            w_cy = [[wusb[:, 2 * k + n, :] for n in range(NDIM)] for k in range(4)]
            w_q = [[wpsb[:, 2 * (r - 1) + n, :] for n in range(NDIM)]
                   for r in (1, 2, 3)]
            w_cw = wpsb[:, 6, :]
            w_cqs = wpsb[:, 7, :]
            w_cq2s = wpsb[:, 8, :]
            q4b = [csb[:, n : n + 1].to_broadcast([D_PER, J]) for n in range(NDIM)]

            def emit_u(b):
                """u_n matmuls into PSUM + DVE scans -> Y'_n (c_n folded)."""
                xu = xus[b]
                # phase-0's x-term only needs xu: pack early on the DVE queue
                t0 = ysp.tile([D_PER, J], BF16, tag="t0")
                nc.vector.tensor_scalar_mul(out=t0[:, :], in0=xu[:, 0, :],
                                            scalar1=csb[:, 2:3])
                t1 = None
                if b == BSZ - 1:
                    # last batch: phase 1 runs on the DVE (shorter PE tail)
                    t1 = ysp.tile([D_PER, J], BF16, tag="t1")
                    nc.vector.tensor_scalar_mul(out=t1[:, :], in0=xu[:, 1, :],
                                                scalar1=csb[:, 5:6])
                Y = []
                for n in range(NDIM):
                    pu = psu.tile([D_PER, J], F32, tag="u")
                    # k-outer so both j-groups reuse one stationary weight
                    # back-to-back (halves the LDWEIGHTS load on the PE NX)
                    for k in range(4):
                        for g in range(NG):
                            s = bass.ts(g, CH)
                            if k == 0:
                                nc.tensor.matmul(pu[:, s], w_cy[0][n],
                                                 xu[:, 0, s],
                                                 start=True, stop=False)
                            elif g == 0:
                                nc.tensor.matmul(
                                    pu[:, 1:CH], w_cy[k][n],
                                    xu[:, 4 - k, 0 : CH - 1],
                                    start=False, stop=(k == 3))
                            else:
                                nc.tensor.matmul(
                                    pu[:, s], w_cy[k][n],
                                    xu[:, 4 - k, g * CH - 1 : (g + 1) * CH - 1],
                                    start=False, stop=(k == 3))
                    yn = yp.tile([D_PER, J], BF16, tag=f"y{n}")
                    nc.vector.tensor_tensor_scan(
                        out=yn[:, :], data0=q4b[n], data1=pu[:, :],
                        initial=0.0, op0=ALU.mult, op1=ALU.add,
                    )
                    Y.append(yn)
                return t0, t1, Y

            def emit_phases(b, t0, t1, Y):
                xu = xus[b]
                ob = op.tile([D_PER, DEC, J], BF16)

                # --- phase 0 on DVE: pre0 = w*x0 + Y'0 + Y'1
                a0 = ysp.tile([D_PER, J], BF16, tag="a0")
                nc.vector.tensor_add(out=a0[:, :], in0=t0[:, :], in1=Y[0][:, :])
                pre0 = ysp.tile([D_PER, J], BF16, tag="pre0")
                nc.vector.tensor_add(out=pre0[:, :], in0=a0[:, :], in1=Y[1][:, :])
                nc.scalar.activation(out=ob[:, 0, :], in_=pre0[:, :], func=AF.Silu)
                nc.sync.dma_start(out=out[:, b, 0, :], in_=ob[:, 0, :])

                pe_phases = (1, 2, 3)
                if t1 is not None:
                    # last batch: pre1 = csum*x1 + q0 Y'0 + q1 Y'1 on the
                    # otherwise-idle DVE, overlapping the r2/r3 matmuls
                    b0 = ysp.tile([D_PER, J], BF16, tag="b0")
                    nc.vector.tensor_scalar_mul(out=b0[:, :], in0=Y[0][:, :],
                                                scalar1=csb[:, 3:4])
                    b1 = ysp.tile([D_PER, J], BF16, tag="b1")
                    nc.vector.scalar_tensor_tensor(
                        out=b1[:, :], in0=Y[1][:, :], scalar=csb[:, 4:5],
                        in1=b0[:, :], op0=ALU.mult, op1=ALU.add)
                    pre1 = ysp.tile([D_PER, J], BF16, tag="pre1")
                    nc.vector.tensor_add(out=pre1[:, :], in0=t1[:, :],
                                         in1=b1[:, :])
                    nc.scalar.activation(out=ob[:, 1, :], in_=pre1[:, :],
                                         func=AF.Silu)
                    nc.scalar.dma_start(out=out[:, b, 1, :], in_=ob[:, 1, :])
                    pe_phases = (2, 3)

                # --- remaining phases on PE: x-terms + Y-terms in PSUM
                # weight-outer over the two j-groups: each stationary weight
                # serves 2 consecutive matmuls. For the last batch the
                # phases are interleaved stage-wise (all x-terms first, then
                # Y0-terms, then Y1-terms) so the scan-n1 wait is filled
                # with x-term matmuls.
                def xw_of(r):
                    xw = [(w_cw, r)]
                    if r == 2:
                        xw.append((w_cqs, 1))
                    elif r == 3:
                        xw.append((w_cqs, 2))
                        xw.append((w_cq2s, 1))
                    return xw

                pts = {}
                for r in pe_phases:
                    # last batch: take PSUM from the u-pool, whose banks free
                    # right after the final scans -- earlier than the psc
                    # ring, which waits on batch b-1's silu reads
                    if t1 is not None:
                        pt_r = psu.tile([D_PER, J], F32, tag="u")
                    else:
                        pt_r = psc.tile([D_PER, J], F32, tag="pt")
                    pts[r] = pt_r
                interleave = t1 is not None
                stages = []
                if interleave:
                    stages.append([(r, "x") for r in pe_phases])
                    stages.append([(r, n) for n in range(NDIM)
                                   for r in pe_phases])
                else:
                    stages.append([(r, st) for r in pe_phases
                                   for st in ("x", 0, 1)])
                for stage in stages:
                    for r, st in stage:
                        pt = pts[r]
                        if st == "x":
                            for i, (wt, rr) in enumerate(xw_of(r)):
                                for g in range(NG):
                                    s = bass.ts(g, CH)
                                    nc.tensor.matmul(pt[:, s], wt,
                                                     xu[:, rr, s],
                                                     start=(i == 0),
                                                     stop=False)
                        else:
                            n = st
                            for g in range(NG):
                                s = bass.ts(g, CH)
                                nc.tensor.matmul(pt[:, s], w_q[r - 1][n],
                                                 Y[n][:, s], start=False,
                                                 stop=(n == NDIM - 1))
                            if n == NDIM - 1:
                                nc.scalar.activation(out=ob[:, r, :],
                                                     in_=pt[:, :],
                                                     func=AF.Silu)
                                if interleave and r == 3:
                                    # final transfer: halves on both rings
                                    nc.sync.dma_start(
                                        out=out[:, b, r, 0:CH],
                                        in_=ob[:, r, 0:CH])
                                    nc.scalar.dma_start(
                                        out=out[:, b, r, CH:J],
                                        in_=ob[:, r, CH:J])
                                else:
                                    eng = nc.sync if r == 2 else nc.scalar
                                    eng.dma_start(out=out[:, b, r, :],
                                                  in_=ob[:, r, :])

            # software pipeline: u(b+1) is queued on PE before phases(b) so
            # the PE never stalls waiting for batch b's scans
            pend = emit_u(0)
            for b in range(BSZ):
                nxt = emit_u(b + 1) if b + 1 < BSZ else None
                emit_phases(b, *pend)
                pend = nxt

    nc.compile()
    return nc


_CACHE: dict = {}


def _get_nc():
    if "nc" not in _CACHE:
        _CACHE["nc"] = build_bass()
    return _CACHE["nc"]


def make_in_maps(inputs):
    x = np.asarray(inputs["x"], np.float32)
    delta = np.asarray(inputs["delta"], np.float64).reshape(EMBED_DIM, NDIM)
    alpha = np.asarray(inputs["alpha"], np.float64).reshape(EMBED_DIM, NDIM)
    beta = np.asarray(inputs["beta"], np.float64).reshape(EMBED_DIM, NDIM)
    gamma = np.asarray(inputs["gamma"], np.float64).reshape(EMBED_DIM, NDIM)
    omega = np.asarray(inputs["omega"], np.float64).reshape(EMBED_DIM, 1)

    p = 1.0 / (1.0 + np.exp(-delta))
    q = 1.0 - p / (1.0 + np.exp(-alpha))          # [D, 2]
    c = p * beta * gamma * SCALE                   # [D, 2]
    csum = (c.sum(1) + omega[:, 0])[:, None]

    wu_vals = np.concatenate([c * q**k for k in range(4)], axis=1)
    wp_vals = np.concatenate(
        [q, q**2, q**3, csum, (c * q).sum(1)[:, None],
         (c * q**2).sum(1)[:, None]], axis=1)
    coef_full = np.concatenate([q**4, omega, q, csum], axis=1).astype(np.float32)
    assert wu_vals.shape == (EMBED_DIM, NWU)
    assert wp_vals.shape == (EMBED_DIM, NWP)
    assert coef_full.shape == (EMBED_DIM, NCOEF)

    in_maps = []
    idx = np.arange(D_PER)
    for c_ in range(N_CORES):
        sl = slice(c_ * D_PER, (c_ + 1) * D_PER)
        xc = x[:, :, sl].transpose(2, 1, 0).astype(ml_dtypes.bfloat16)  # [128,B,L]
        ph = xc.reshape(D_PER, BSZ, J, DEC).transpose(0, 1, 3, 2)  # [128,B,4,J]
        wuc = np.zeros((D_PER, NWU, D_PER), dtype=ml_dtypes.bfloat16)
        wuc[idx, :, idx] = wu_vals[sl].astype(ml_dtypes.bfloat16)
        wpc = np.zeros((D_PER, NWP, D_PER), dtype=ml_dtypes.bfloat16)
        wpc[idx, :, idx] = wp_vals[sl].astype(ml_dtypes.bfloat16)
        in_maps.append(
            {"x": np.ascontiguousarray(ph),
             "wdu": wuc, "wdp": wpc,
             "coef": np.ascontiguousarray(coef_full[sl])}
        )
    return in_maps


def gather_out(results):
    out = np.empty((SEQ_LEN, BSZ, EMBED_DIM), np.float32)
    for c in range(N_CORES):
        # [128, B, 4, J] phase-major -> [l = 4j+r, b, d]
        arr = results[c]["out"].astype(np.float32)
        out[:, :, c * D_PER : (c + 1) * D_PER] = arr.transpose(3, 2, 1, 0).reshape(
            SEQ_LEN, BSZ, D_PER
        )
    return out


def _run(inputs, **kwargs):
    nc = _get_nc()
    in_maps = make_in_maps(inputs)
    res = run_bass_kernel_spmd(nc, in_maps, core_ids=list(range(N_CORES)), **kwargs)
    return gather_out(res.results), res


def kernel(**inputs) -> np.ndarray:
    out, _ = _run(inputs)
    return out
